# revision 7
# baseline (speedup 1.0000x reference)
"""Trainium2 Bass kernel for nn_DihedralGroupConv.

Math: reference computes
    filt[c,i,d,o] = sum_g perm[g,i,o] * weight[g,c,d]
    out = x.reshape(B,-1) @ filt.reshape(C*2n, D*2n)
i.e. out[b,d,o] = sum_{g,c} weight[g,c,d] * x[b,c, idx_g(o)]
where perm[g] are permutation matrices of the dihedral regular
representation: each is a half-wise cyclic shift of either x itself
(rotations) or of the reflected array xr (reflections).

Kernel strategy (data-parallel over batch, 64 b per core):
  - Host precomputes a halo-padded (216 = 200 + 2*8) per-half image of x,
    laid out as the SBUF image AX[128, 16, 2, 216] with partition =
    32*(b%4) + c.  Rotations read a shifted window forward, reflections
    read a shifted window BACKWARD (negative-stride access pattern with
    swapped halves).
  - Weights are staged as one 128x128 BLOCK-DIAGONAL stationary image
    per generator (W_g replicated on the 4 diagonal 32x32 blocks), so a
    single full-array matmul per (quad, gen) computes all 4 batch
    elements of the quad at once: 64 matmuls of 400 moving columns.
    The 128-column stationary loads take the Fast-Weight-Load path and
    pipeline into the background weight buffer under the previous
    matmul's stream.
  - Quads are processed in groups of 4 (one PSUM bank per quad);
    generator contributions accumulate in PSUM (start/stop flags);
    groups ping-pong between bank sets 0-3 and 4-7 so one group's
    PSUM drain (DVE+ACT f32->fp16 copies) hides under the next
    group's matmul stream.  Output DMAs alternate between the two
    HWDGE rings (sync / scalar); the last group's output is split in
    two so the final transfer starts as early as possible.
  - Warm-up: dummy matmuls (zero tile memset on the DVE, whose queue is
    free at startup) keep the PE busy from engine start until the first
    data chunk lands, so the HAM clock gate is released (2.4 GHz) when
    the real stream begins.
All DMAs are pure 128-partition contiguous-run transfers.
"""

import numpy as np

import concourse.bass as bass  # noqa: F401  (kept for users of this module)
import concourse.mybir as mybir
from concourse import bacc
from concourse.tile import TileContext
from concourse.bass_utils import run_bass_kernel_spmd

# Problem constants (hardcoded per harness contract).
B = 512
C = 32          # in channels
D = 32          # out channels
N = 200         # half length; 2N = 400
L = 2 * N
N_CORES = 8
BPC = B // N_CORES          # 64 batch per core
NQ = BPC // 4               # 16 quads (4 batch / quad)
HALO = 8
PH = N + 2 * HALO           # 216 padded half length
GRP = 4                     # quads per PSUM group
NDUMMY = 11                 # warm-up matmuls (~3.7us at 1.2 GHz)

_DT_IN = mybir.dt.float16   # 1 cyc/col PE mode, 1-pass weight load, half DMA
_DT_OUT = mybir.dt.float16  # output staged/stored as fp16, host casts to f32
_NP_IN = np.float16

_cache = {}


def _derive_gens(perm):
    """Classify each generator as (is_refl, shift s) with y[o] = base[(o+s)%N]
    per half, where base is x (rotation) or xr (reflection)."""
    n = N
    o = np.arange(n)
    gens = []
    for g in range(perm.shape[0]):
        idx = np.argmax(perm[g], axis=0).astype(np.int64)  # y[o] = x[idx[o]]
        # rotation candidate: idx[o] = (o - r) % n ; idx[n+o] = n + (o-r)%n
        r = int((-idx[0]) % n)
        rot = np.concatenate([(o - r) % n, n + (o - r) % n])
        if np.array_equal(idx, rot):
            s = -r if r <= n // 2 else n - r
            gens.append((False, s))
            continue
        # reflection candidate: y[o] = xr[(o+r)%n per half] with
        # xr[t] = x[n + (-t)%n], xr[n+t] = x[(-t)%n]
        # => idx[o] = n + (-o-r)%n ; idx[n+o] = (-o-r)%n
        r = int(idx[0] - n) % n     # idx[0] = n + (-r)%n -> (-r)%n
        r = (-r) % n
        refl = np.concatenate([n + (-o - r) % n, (-o - r) % n])
        if np.array_equal(idx, refl):
            s = r if r <= n // 2 else r - n
            gens.append((True, s))
            continue
        raise NotImplementedError(f"perm[{g}] is not a dihedral rep matrix")
    for is_refl, s in gens:
        if is_refl:
            ok = -(HALO - 1) <= s <= HALO
        else:
            ok = -HALO <= s <= HALO
        if not ok:
            raise NotImplementedError(f"shift {s} exceeds halo {HALO}")
    return gens


def _build_program(gens):
    """Build + compile the SPMD Bass program (identical on all cores)."""
    ng = len(gens)

    nc = bacc.Bacc("TRN2", target_bir_lowering=False, debug=False,
                   num_devices=N_CORES, enable_partition_id=False)
    ax_d = nc.dram_tensor("ax", [128, NQ, 2, PH], _DT_IN,
                          kind="ExternalInput")
    ws_d = nc.dram_tensor("ws", [128, 128 * ng], _DT_IN,
                          kind="ExternalInput")
    outr_d = nc.dram_tensor("outr", [128, NQ * L], _DT_OUT,
                            kind="ExternalOutput")

    CHUNKS = [1, 1, 2, 4, 4, 4]  # DMA chunk sizes along quads (small first
                                 # chunks -> matmuls start earlier)
    with TileContext(nc) as tc:
        with (
            tc.tile_pool(name="arrp", bufs=1) as arrp,
            tc.tile_pool(name="wsp", bufs=1) as wsp,
            tc.tile_pool(name="stg", bufs=1) as stgp,
            tc.tile_pool(name="psum", bufs=1, space="PSUM") as psump,
        ):
            ws_sb = wsp.tile([128, 128 * ng], _DT_IN)
            ax_sb = arrp.tile([128, NQ, 2, PH], _DT_IN, name="ax_sb")
            # weights go on the scalar HWDGE queue so the first data chunk
            # and the weights transfer run concurrently; generator 0's
            # block goes first so the opening matmul is not gated on the
            # full weight image
            nc.scalar.dma_start(out=ws_sb[:, 0:128], in_=ws_d[:, 0:128])
            nc.scalar.dma_start(out=ws_sb[:, 128:128 * ng],
                                in_=ws_d[:, 128:128 * ng])
            c0 = 0
            for cq in CHUNKS:
                nc.sync.dma_start(out=ax_sb[:, c0:c0 + cq],
                                  in_=ax_d[:, c0:c0 + cq])
                c0 += cq

            # two 4-bank PSUM mega-tiles (banks 0-3 / 4-7); groups of GRP
            # quads ping-pong between them.  512-f32 pitch aligns each
            # quad's 400 columns to its own bank.
            psA = psump.tile([128, GRP, 512], mybir.dt.float32, name="psA")
            psB = psump.tile([128, GRP, 512], mybir.dt.float32, name="psB")
            stgs = [stgp.tile([128, GRP, L], _DT_OUT, name=f"stg{i}")
                    for i in range(2)]

            # HAM warm-up: dummy matmuls keep the PE busy while the first
            # data chunk is in flight, so the clock gate is already
            # released (2.4 GHz) when the real stream starts.  Zero-fill
            # on the DVE (its queue is free at startup, unlike gpsimd);
            # the PSUM scribbles are overwritten by the first start=True
            # matmul of each bank.
            wu_sb = wsp.tile([128, L], _DT_IN, name="wu_sb")
            nc.vector.memset(wu_sb[:, :], 0.0)
            for i in range(NDUMMY):
                ps = psA if i % 2 == 0 else psB
                nc.tensor.matmul(ps[:, (i // 2) % GRP, 0:L],
                                 wu_sb[:, 0:128], wu_sb[:, :],
                                 start=True, stop=True)

            # per (quad, gen): ONE 128-wide block-diagonal matmul
            # (batch element u -> SBUF rows 32u -> PSUM partitions 32u).
            # (is_refl, weight col block, window param)
            mm_descs = []
            for j, (is_r, s) in enumerate(gens):
                if not is_r:
                    mm_descs.append((False, 128 * j, s + HALO))
                else:
                    mm_descs.append((True, 128 * j, s))

            axt = ax_sb[:, :, :, :]
            pstride = axt.ap[0][0]      # free elems per partition

            def rhs_ap(q, is_r, w):
                if not is_r:
                    return ax_sb[:, q, :, w:w + N]
                # reflection: swapped halves, backward o scan;
                # out (h, o) reads src[1-h, (HALO+200) - o - s]
                off = q * (2 * PH) + PH + (PH - HALO - w)
                return bass.AP(axt.tensor, off,
                               [[pstride, 128], [-PH, 2], [-1, N]])

            ngrp = NQ // GRP
            for grp in range(ngrp):
                ps = psA if grp % 2 == 0 else psB
                stg = stgs[grp % 2]
                base = GRP * grp * L
                last = grp == ngrp - 1
                if not last:
                    # gen-major: one weight load serves GRP matmuls
                    for gi, (is_r, wc, w) in enumerate(mm_descs):
                        for qi in range(GRP):
                            q = GRP * grp + qi
                            nc.tensor.matmul(
                                ps[:, qi, 0:L],
                                ws_sb[:, wc:wc + 128],
                                rhs_ap(q, is_r, w),
                                start=(gi == 0), stop=(gi == ng - 1),
                            )
                    # bank-pair drains: one wide copy per engine
                    nc.vector.tensor_copy(out=stg[:, 0:2, :],
                                          in_=ps[:, 0:2, 0:L])
                    nc.scalar.copy(out=stg[:, 2:4, :], in_=ps[:, 2:4, 0:L])
                    eng = nc.sync if grp % 2 == 0 else nc.scalar
                    eng.dma_start(out=outr_d[:, base:base + GRP * L],
                                  in_=stg[:, :, :])
                else:
                    # quad-major last group: each quad's accumulation
                    # finishes early, so its drain + small output DMA
                    # overlap the remaining quads' matmuls -> short tail
                    for qi in range(GRP):
                        q = GRP * grp + qi
                        for gi, (is_r, wc, w) in enumerate(mm_descs):
                            nc.tensor.matmul(
                                ps[:, qi, 0:L],
                                ws_sb[:, wc:wc + 128],
                                rhs_ap(q, is_r, w),
                                start=(gi == 0), stop=(gi == ng - 1),
                            )
                        if qi % 2 == 0:
                            nc.vector.tensor_copy(out=stg[:, qi, :],
                                                  in_=ps[:, qi, 0:L])
                        else:
                            nc.scalar.copy(out=stg[:, qi, :],
                                           in_=ps[:, qi, 0:L])
                        eng = nc.sync if qi % 2 == 0 else nc.scalar
                        eng.dma_start(
                            out=outr_d[:, base + qi * L:base + (qi + 1) * L],
                            in_=stg[:, qi, :])
    nc.compile()
    return nc


def _host_images(x, weight, gens):
    """Build per-core AX images and the packed block-diag weight image."""
    n = N
    ng = len(gens)

    pad_idx = (np.arange(PH) - HALO) % n
    xh = x.reshape(B, C, 2, n)[:, :, :, pad_idx]          # [B, C, 2, PH]

    ws = np.zeros((128, 128 * ng), dtype=_NP_IN)
    for g in range(ng):
        for u in range(4):
            ws[32 * u:32 * (u + 1),
               128 * g + 32 * u:128 * g + 32 * (u + 1)] = weight[g]

    def img(a, core):
        sl = a[core * BPC:(core + 1) * BPC]               # [64, C, 2, PH]
        out = np.empty((128, NQ, 2, PH), dtype=_NP_IN)
        for u in range(4):
            out[32 * u:32 * (u + 1)] = sl[u::4].transpose(1, 0, 2, 3)
        return np.ascontiguousarray(out)

    axs = [img(xh, c) for c in range(N_CORES)]
    return axs, ws


def _unscramble(outr):
    """outr[32*(b%4)+d, (b>>2)*L + o] -> out shard [BPC, D, L]."""
    r = outr.astype(np.float32).reshape(4, D, NQ, L)    # [b%4, d, q, o]
    r = r.transpose(2, 0, 1, 3)                         # [q, b%4, d, o]
    return np.ascontiguousarray(r.reshape(BPC, D, L))


def kernel(x, weight, perm, _trace=False):
    x = np.asarray(x, dtype=np.float32)
    weight = np.asarray(weight, dtype=np.float32)
    perm = np.asarray(perm, dtype=np.float32)

    gens = _derive_gens(perm)
    key = tuple(gens)
    if key not in _cache:
        _cache[key] = _build_program(gens)
    nc = _cache[key]

    axs, ws = _host_images(x, weight, gens)
    in_maps = [{"ax": axs[c], "ws": ws} for c in range(N_CORES)]
    res = run_bass_kernel_spmd(nc, in_maps, core_ids=list(range(N_CORES)),
                               trace=_trace)
    out = np.concatenate([_unscramble(res.results[c]["outr"])
                          for c in range(N_CORES)], axis=0)
    if _trace:
        kernel.last_exec_time_ns = res.exec_time_ns
        kernel.last_results = res
    return out


# revision 10
# speedup vs baseline: 1.2786x; 1.2786x over previous
"""Trainium2 Bass kernel for nn_DihedralGroupConv.

Math: reference computes
    filt[c,i,d,o] = sum_g perm[g,i,o] * weight[g,c,d]
    out = x.reshape(B,-1) @ filt.reshape(C*2n, D*2n)
i.e. out[b,d,o] = sum_{g,c} weight[g,c,d] * x[b,c, idx_g(o)]
where perm[g] are permutation matrices of the dihedral regular
representation: each is a half-wise cyclic shift of either x itself
(rotations) or of the reflected array xr (reflections).

Kernel strategy (data-parallel over batch, 64 b per core):
  - Host precomputes a halo-padded (216 = 200 + 2*8) per-half image of x,
    laid out as the SBUF image AX[128, 16, 2, 216] with partition =
    32*(b%4) + c.  Rotations read a shifted window forward, reflections
    read a shifted window BACKWARD (negative-stride access pattern with
    swapped halves).
  - Weights are staged as one 128x128 BLOCK-DIAGONAL stationary image
    per generator (W_g replicated on the 4 diagonal 32x32 blocks), so a
    single full-array matmul per (quad, gen) computes all 4 batch
    elements of the quad at once: 64 matmuls of 400 moving columns.
    The 128-column stationary loads take the Fast-Weight-Load path and
    pipeline into the background weight buffer under the previous
    matmul's stream.
  - Quads are processed in groups of 4 (one PSUM bank per quad);
    generator contributions accumulate in PSUM (start/stop flags);
    groups ping-pong between bank sets 0-3 and 4-7 so one group's
    PSUM drain (DVE+ACT f32->fp16 copies) hides under the next
    group's matmul stream.  Output DMAs alternate between the two
    HWDGE rings (sync / scalar); the last group's output is split in
    two so the final transfer starts as early as possible.
  - Warm-up: dummy matmuls (zero tile memset on the DVE, whose queue is
    free at startup) keep the PE busy from engine start until the first
    data chunk lands, so the HAM clock gate is released (2.4 GHz) when
    the real stream begins.
All DMAs are pure 128-partition contiguous-run transfers.
"""

import numpy as np

import concourse.bass as bass  # noqa: F401  (kept for users of this module)
import concourse.mybir as mybir
from concourse import bacc
from concourse.tile import TileContext
from concourse.bass_utils import run_bass_kernel_spmd

# Problem constants (hardcoded per harness contract).
B = 512
C = 32          # in channels
D = 32          # out channels
N = 200         # half length; 2N = 400
L = 2 * N
N_CORES = 8
BPC = B // N_CORES          # 64 batch per core
NQ = BPC // 4               # 16 quads (4 batch / quad)
HALO = 8
PH = N + 2 * HALO           # 216 padded half length
GRP = 4                     # quads per PSUM group
NDUMMY = 8                  # warm-up matmuls (~2.7us at 1.2 GHz)

_DT_IN = mybir.dt.float16   # 1 cyc/col PE mode, 1-pass weight load, half DMA
_DT_OUT = mybir.dt.float16  # output staged/stored as fp16, host casts to f32
_NP_IN = np.float16

_cache = {}


def _derive_gens(perm):
    """Classify each generator as (is_refl, shift s) with y[o] = base[(o+s)%N]
    per half, where base is x (rotation) or xr (reflection)."""
    n = N
    o = np.arange(n)
    gens = []
    for g in range(perm.shape[0]):
        idx = np.argmax(perm[g], axis=0).astype(np.int64)  # y[o] = x[idx[o]]
        # rotation candidate: idx[o] = (o - r) % n ; idx[n+o] = n + (o-r)%n
        r = int((-idx[0]) % n)
        rot = np.concatenate([(o - r) % n, n + (o - r) % n])
        if np.array_equal(idx, rot):
            s = -r if r <= n // 2 else n - r
            gens.append((False, s))
            continue
        # reflection candidate: y[o] = xr[(o+r)%n per half] with
        # xr[t] = x[n + (-t)%n], xr[n+t] = x[(-t)%n]
        # => idx[o] = n + (-o-r)%n ; idx[n+o] = (-o-r)%n
        r = int(idx[0] - n) % n     # idx[0] = n + (-r)%n -> (-r)%n
        r = (-r) % n
        refl = np.concatenate([n + (-o - r) % n, (-o - r) % n])
        if np.array_equal(idx, refl):
            s = r if r <= n // 2 else r - n
            gens.append((True, s))
            continue
        raise NotImplementedError(f"perm[{g}] is not a dihedral rep matrix")
    for is_refl, s in gens:
        if is_refl:
            ok = -(HALO - 1) <= s <= HALO
        else:
            ok = -HALO <= s <= HALO
        if not ok:
            raise NotImplementedError(f"shift {s} exceeds halo {HALO}")
    return gens


def _build_program(gens):
    """Build + compile the SPMD Bass program (identical on all cores)."""
    ng = len(gens)

    nc = bacc.Bacc("TRN2", target_bir_lowering=False, debug=False,
                   num_devices=N_CORES, enable_partition_id=False)
    ax_d = nc.dram_tensor("ax", [128, NQ, 2, PH], _DT_IN,
                          kind="ExternalInput")
    ws_d = nc.dram_tensor("ws", [128, 128 * ng], _DT_IN,
                          kind="ExternalInput")
    outr_d = nc.dram_tensor("outr", [128, NQ * L], _DT_OUT,
                            kind="ExternalOutput")

    CHUNKS = [2, 2, 4, 4, 4]    # DMA chunk sizes along quads (small first
                                # chunks -> matmuls start earlier)
    with TileContext(nc) as tc:
        with (
            tc.tile_pool(name="arrp", bufs=1) as arrp,
            tc.tile_pool(name="wsp", bufs=1) as wsp,
            tc.tile_pool(name="stg", bufs=1) as stgp,
            tc.tile_pool(name="psum", bufs=1, space="PSUM") as psump,
        ):
            ws_sb = wsp.tile([128, 128 * ng], _DT_IN)
            ax_sb = arrp.tile([128, NQ, 2, PH], _DT_IN, name="ax_sb")
            # weights go on the scalar HWDGE queue so the first data chunk
            # and the weights transfer run concurrently; gens 0-1 first so
            # the opening matmuls are not gated on the full weight image
            nc.scalar.dma_start(out=ws_sb[:, 0:256], in_=ws_d[:, 0:256])
            nc.scalar.dma_start(out=ws_sb[:, 256:128 * ng],
                                in_=ws_d[:, 256:128 * ng])
            c0 = 0
            for cq in CHUNKS:
                nc.sync.dma_start(out=ax_sb[:, c0:c0 + cq],
                                  in_=ax_d[:, c0:c0 + cq])
                c0 += cq

            # one PSUM bank per quad-in-flight; groups of GRP quads
            # ping-pong between the two bank halves
            pstiles = [psump.tile([128, L], mybir.dt.float32,
                                  name=f"ps{i}") for i in range(2 * GRP)]
            stgs = [stgp.tile([128, GRP, L], _DT_OUT, name=f"stg{i}")
                    for i in range(2)]

            # HAM warm-up: dummy matmuls keep the PE busy while the first
            # data chunk is in flight, so the clock gate is already
            # released (2.4 GHz) when the real stream starts.  Zero-fill
            # on the DVE (its queue is free at startup, unlike gpsimd);
            # the PSUM scribbles are overwritten by the first start=True
            # matmul of each bank.
            wu_sb = wsp.tile([128, L], _DT_IN, name="wu_sb")
            nc.vector.memset(wu_sb[:, :], 0.0)
            for i in range(NDUMMY):
                nc.tensor.matmul(pstiles[i % (2 * GRP)][:, :],
                                 wu_sb[:, 0:128], wu_sb[:, :],
                                 start=True, stop=True)

            # per (quad, gen): ONE 128-wide block-diagonal matmul
            # (batch element u -> SBUF rows 32u -> PSUM partitions 32u).
            # (is_refl, weight col block, window param)
            mm_descs = []
            for j, (is_r, s) in enumerate(gens):
                if not is_r:
                    mm_descs.append((False, 128 * j, s + HALO))
                else:
                    mm_descs.append((True, 128 * j, s))

            axt = ax_sb[:, :, :, :]
            pstride = axt.ap[0][0]      # free elems per partition

            def rhs_ap(q, is_r, w):
                if not is_r:
                    return ax_sb[:, q, :, w:w + N]
                # reflection: swapped halves, backward o scan;
                # out (h, o) reads src[1-h, (HALO+200) - o - s]
                off = q * (2 * PH) + PH + (PH - HALO - w)
                return bass.AP(axt.tensor, off,
                               [[pstride, 128], [-PH, 2], [-1, N]])

            ngrp = NQ // GRP
            for grp in range(ngrp):
                pss = [pstiles[GRP * (grp % 2) + i] for i in range(GRP)]
                stg = stgs[grp % 2]
                base = GRP * grp * L
                last = grp == ngrp - 1
                if not last:
                    # gen-major: one weight load serves GRP matmuls
                    for gi, (is_r, wc, w) in enumerate(mm_descs):
                        for qi in range(GRP):
                            q = GRP * grp + qi
                            nc.tensor.matmul(
                                pss[qi][:, :],
                                ws_sb[:, wc:wc + 128],
                                rhs_ap(q, is_r, w),
                                start=(gi == 0), stop=(gi == ng - 1),
                            )
                    for qi in range(GRP):
                        if qi < GRP // 2:
                            nc.vector.tensor_copy(out=stg[:, qi],
                                                  in_=pss[qi][:, :])
                        else:
                            nc.scalar.copy(out=stg[:, qi],
                                           in_=pss[qi][:, :])
                    eng = nc.sync if grp % 2 == 0 else nc.scalar
                    eng.dma_start(out=outr_d[:, base:base + GRP * L],
                                  in_=stg[:, :, :])
                else:
                    # quad-major last group: each quad's accumulation
                    # finishes early, so its drain + small output DMA
                    # overlap the remaining quads' matmuls -> short tail
                    for qi in range(GRP):
                        q = GRP * grp + qi
                        for gi, (is_r, wc, w) in enumerate(mm_descs):
                            nc.tensor.matmul(
                                pss[qi][:, :],
                                ws_sb[:, wc:wc + 128],
                                rhs_ap(q, is_r, w),
                                start=(gi == 0), stop=(gi == ng - 1),
                            )
                        if qi % 2 == 0:
                            nc.vector.tensor_copy(out=stg[:, qi, :],
                                                  in_=pss[qi][:, :])
                        else:
                            nc.scalar.copy(out=stg[:, qi, :],
                                           in_=pss[qi][:, :])
                        eng = nc.sync if qi % 2 == 0 else nc.scalar
                        eng.dma_start(
                            out=outr_d[:, base + qi * L:base + (qi + 1) * L],
                            in_=stg[:, qi, :])
    nc.compile()
    return nc


def _host_images(x, weight, gens):
    """Build per-core AX images and the packed block-diag weight image."""
    n = N
    ng = len(gens)

    pad_idx = (np.arange(PH) - HALO) % n
    xh = x.reshape(B, C, 2, n)[:, :, :, pad_idx]          # [B, C, 2, PH]

    ws = np.zeros((128, 128 * ng), dtype=_NP_IN)
    for g in range(ng):
        for u in range(4):
            ws[32 * u:32 * (u + 1),
               128 * g + 32 * u:128 * g + 32 * (u + 1)] = weight[g]

    def img(a, core):
        sl = a[core * BPC:(core + 1) * BPC]               # [64, C, 2, PH]
        out = np.empty((128, NQ, 2, PH), dtype=_NP_IN)
        for u in range(4):
            out[32 * u:32 * (u + 1)] = sl[u::4].transpose(1, 0, 2, 3)
        return np.ascontiguousarray(out)

    axs = [img(xh, c) for c in range(N_CORES)]
    return axs, ws


def _unscramble(outr):
    """outr[32*(b%4)+d, (b>>2)*L + o] -> out shard [BPC, D, L]."""
    r = outr.astype(np.float32).reshape(4, D, NQ, L)    # [b%4, d, q, o]
    r = r.transpose(2, 0, 1, 3)                         # [q, b%4, d, o]
    return np.ascontiguousarray(r.reshape(BPC, D, L))


def kernel(x, weight, perm, _trace=False):
    x = np.asarray(x, dtype=np.float32)
    weight = np.asarray(weight, dtype=np.float32)
    perm = np.asarray(perm, dtype=np.float32)

    gens = _derive_gens(perm)
    key = tuple(gens)
    if key not in _cache:
        _cache[key] = _build_program(gens)
    nc = _cache[key]

    axs, ws = _host_images(x, weight, gens)
    in_maps = [{"ax": axs[c], "ws": ws} for c in range(N_CORES)]
    res = run_bass_kernel_spmd(nc, in_maps, core_ids=list(range(N_CORES)),
                               trace=_trace)
    out = np.concatenate([_unscramble(res.results[c]["outr"])
                          for c in range(N_CORES)], axis=0)
    if _trace:
        kernel.last_exec_time_ns = res.exec_time_ns
        kernel.last_results = res
    return out
